# revision 1
# baseline (speedup 1.0000x reference)
"""Trainium2 Bass kernel for the BSG word2gauss-style hinge/KL loss.

Strategy (data-parallel over 8 NeuronCores):
  - Host precomputes gather tables (batch-independent weight prep), bf16:
      UT [V,128] : cols 0:50 = U    = emb @ W1[:50]   (context embed path)
      CU [V,128] : cols 0:50 = Ucen = emb @ W1[50:] + b1 (center path)
      TT [V,128] : cols 0:100 = type_means, 100 = logvar, 101 = sum(m^2),
                   102 = exp(-logvar)
  - Gathers use dma_gather (SWDGE). Its int16 index limit (<32768 rows) is
    handled by gathering PAIRED rows: index = id>>1 with elem_size = 2 rows
    (512B), then a parity select keeps the right half. <=1024 indices per
    instruction (SWDGE descriptor-ring capacity), spread over 4 queues.
  - Each core processes 8192 batch rows in 16 gather-blocks of 512. Flat
    gather position i -> (partition i%128, slot i//128), so host index
    order is slot-major. Per 128-row sub-block:
      h = sum_j relu(U[ctx_j] + Ucen[cen]);  [h;1] @ [Wmu|Wls;bmu|bls] on PE
      A = exp(logsigma) + sum(mu^2);  dot2 = sum(-2mu * m_w) for all rows
    then kl algebra + hinge on [128,40] vectors, accumulated in f32.
  - Output per core: [128,2] partials; host reduces, applies -L/2, /B.
"""

import sys

for _p in ("/opt/trn_rl_repo", "/opt/pypackages"):
    if _p not in sys.path:
        sys.path.append(_p)

from contextlib import ExitStack

import numpy as np
import ml_dtypes

import concourse.bass as bass
import concourse.tile as tile
from concourse import bacc, mybir
from concourse.bass_utils import run_bass_kernel_spmd
from concourse.masks import make_identity

dt = mybir.dt
F32 = dt.float32
BF16 = dt.bfloat16
AF = mybir.ActivationFunctionType
OP = mybir.AluOpType
AX = mybir.AxisListType

V, D, H, L = 50000, 50, 50, 100
C = 10
B = 65536
NCORES = 8
NB = B // NCORES     # rows per core: 8192
GBS = 512            # rows per gather block
NGB = NB // GBS      # 16
NSB = GBS // 128     # 4 sub-blocks
Q = NSB * C          # 40 ctx slots per partition per gather block
E = 128              # table row width (bf16 elems, 256B)
E2 = 2 * E           # paired gather width
MAXI = 1024          # max idxs per dma_gather (SWDGE ring capacity)
MARGIN = 1.0

_CACHE: dict = {}


def _wrap_idx(flat):
    """int16 idx list -> [128, ceil(n/16)] wrapped-16, replicated across cores."""
    n = len(flat)
    nf = -(-n // 16)
    w = np.zeros((16, nf), np.int16)
    w[np.arange(n) % 16, np.arange(n) // 16] = flat
    return np.tile(w, (8, 1))


def _build_program():
    nc = bacc.Bacc("TRN2", target_bir_lowering=False, debug=False, num_swdge_queues=4)

    gt_d = nc.dram_tensor("gt", [V, 2 * E], BF16, kind="ExternalInput")
    cg_d = nc.dram_tensor("cg", [V, 2 * E], BF16, kind="ExternalInput")
    tt_d = nc.dram_tensor("tt", [V, E], BF16, kind="ExternalInput")
    wf_d = nc.dram_tensor("wf", [H + 1, L + 1], F32, kind="ExternalInput")
    # wrapped int16 half-indices, concatenated per gather block:
    #   per gb: ctx (Q*128/16 cols) | neg | cen (NSB*128/16 cols)
    IGC = Q * 128 // 16          # 320 idx cols per gb for ctx/neg streams
    IGZ = NSB * 128 // 16        # 32 idx cols per gb for cen stream
    IG = 2 * IGC + IGZ
    idx_d = nc.dram_tensor("idx", [128, NGB * IG], dt.int16, kind="ExternalInput")
    # parity masks (bf16 0/1): per gb: ctx Q | neg Q | cen NSB
    MG = 2 * Q + NSB
    msk_d = nc.dram_tensor("msk", [128, NGB * MG], dt.uint8, kind="ExternalInput")
    out_d = nc.dram_tensor("out", [128, 2], F32, kind="ExternalOutput")

    # overlapping paired views: row i -> bytes [i*256, i*256+512)
    gt_v = bass.AP(gt_d, 0, [[2 * E2, V // 2], [1, 2 * E2]])
    cg_v = bass.AP(cg_d, 0, [[2 * E2, V // 2], [1, 2 * E2]])
    tt_v = bass.AP(tt_d, 0, [[E2, V // 2], [1, E2]])

    qn = [0]

    def gather(out_ap, tab_v, idx_ap, n, es):
        nc.gpsimd.dma_gather(
            out_ap=out_ap, in_ap=tab_v, idxs_ap=idx_ap,
            num_idxs=n, num_idxs_reg=n, elem_size=es, elem_step=es,
            queue_num=0)

    with tile.TileContext(nc) as tc, ExitStack() as ctx:
        const = ctx.enter_context(tc.tile_pool(name="const", bufs=1))
        io = ctx.enter_context(tc.tile_pool(name="io", bufs=2))
        wk = ctx.enter_context(tc.tile_pool(name="wk", bufs=2))
        ps = ctx.enter_context(tc.tile_pool(name="ps", bufs=2, space="PSUM"))
        accp = ctx.enter_context(tc.tile_pool(name="accp", bufs=1))

        ident = const.tile([128, 128], F32)
        make_identity(nc, ident[:])
        wf_sb = const.tile([H + 1, L + 1], F32)
        nc.sync.dma_start(wf_sb[:], wf_d.ap())
        idx_sb = const.tile([128, NGB * IG], dt.int16)
        nc.sync.dma_start(idx_sb[:], idx_d.ap())
        msk_sb = const.tile([128, NGB * MG], dt.uint8)
        nc.sync.dma_start(msk_sb[:], msk_d.ap())

        acc_h = accp.tile([128, Q], F32)
        acc_c = accp.tile([128, NSB], F32)
        nc.vector.memset(acc_h[:], 0.0)
        nc.vector.memset(acc_c[:], 0.0)

        for gb in range(NGB):
            PG = io.tile([128, Q, 2 * E2], BF16, tag="PG")    # ctx [U|T] pairs
            NT = io.tile([128, Q, E2], BF16, tag="NT")        # neg T pairs
            CG = io.tile([128, NSB, 2 * E2], BF16, tag="CG")  # cen [U|T] pairs

            icx = idx_sb[:, gb * IG:gb * IG + IGC]
            ing = idx_sb[:, gb * IG + IGC:gb * IG + 2 * IGC]
            icn = idx_sb[:, gb * IG + 2 * IGC:(gb + 1) * IG]
            # 1024-idx chunks: chunk g covers slots [g*8, g*8+8)
            NCH = Q * 128 // MAXI                          # 5
            SCH = MAXI // 128                              # 8 slots per chunk
            for g in range(NCH):
                ia = icx[:, g * 64:(g + 1) * 64]           # 1024 idx = 64 cols
                sl = slice(g * SCH, (g + 1) * SCH)
                gather(PG[:, sl, :], gt_v, ia, MAXI, 2 * E2)
                gather(NT[:, sl, :], tt_v, ing[:, g * 64:(g + 1) * 64], MAXI, E2)
            gather(CG[:], cg_v, icn, NSB * 128, 2 * E2)

            # parity select, in place: keep the chosen half in cols [0:E)
            mc = msk_sb[:, gb * MG:gb * MG + Q]
            mn = msk_sb[:, gb * MG + Q:gb * MG + 2 * Q]
            mz = msk_sb[:, gb * MG + 2 * Q:(gb + 1) * MG]
            nc.vector.copy_predicated(PG[:, :, 0:64], mc.unsqueeze(2).to_broadcast([128, Q, 64]),
                                      PG[:, :, E2:E2 + 64])
            nc.vector.copy_predicated(PG[:, :, E:E + 103], mc.unsqueeze(2).to_broadcast([128, Q, 103]),
                                      PG[:, :, E2 + E:E2 + E + 103])
            nc.vector.copy_predicated(NT[:, :, 0:103], mn.unsqueeze(2).to_broadcast([128, Q, 103]),
                                      NT[:, :, E:E + 103])
            nc.vector.copy_predicated(CG[:, :, 0:64], mz.unsqueeze(2).to_broadcast([128, NSB, 64]),
                                      CG[:, :, E2:E2 + 64])
            nc.vector.copy_predicated(CG[:, :, E:E + 103], mz.unsqueeze(2).to_broadcast([128, NSB, 103]),
                                      CG[:, :, E2 + E:E2 + E + 103])

            dc = wk.tile([128, Q], F32, tag="dc")
            dn = wk.tile([128, Q], F32, tag="dn")
            cd = wk.tile([128, NSB], F32, tag="cd")
            A_t = wk.tile([128, NSB], F32, tag="A")
            lsg_t = wk.tile([128, NSB], F32, tag="lsg")

            for s in range(NSB):
                PGs = PG[:, s * C:(s + 1) * C, :]
                NTs = NT[:, s * C:(s + 1) * C, :]

                y = wk.tile([128, C, 64], BF16, tag="y")
                nc.vector.tensor_tensor(
                    out=y[:], in0=PGs[:, :, 0:64],
                    in1=CG[:, s, 0:64].unsqueeze(1).to_broadcast([128, C, 64]),
                    op=OP.add)
                r = wk.tile([128, C, 64], BF16, tag="r")
                nc.scalar.activation(r[:], y[:], AF.Relu)
                h = wk.tile([128, 64], F32, tag="h")
                nc.vector.tensor_reduce(out=h[:], in_=r[:].transpose([0, 2, 1]),
                                        axis=AX.X, op=OP.add)
                nc.vector.memset(h[:, H:H + 1], 1.0)
                hT_ps = ps.tile([64, 128], F32, tag="hTp")
                nc.tensor.transpose(hT_ps[:], h[:], ident[:])
                hT = wk.tile([64, 128], F32, tag="hT")
                nc.scalar.copy(hT[:], hT_ps[:])
                mu_ps = ps.tile([128, L + 1], F32, tag="mu")
                nc.tensor.matmul(mu_ps[:], lhsT=hT[0:H + 1, :], rhs=wf_sb[:],
                                 start=True, stop=True)
                mu2 = wk.tile([128, L], BF16, tag="mu2")
                nc.scalar.mul(mu2[:], mu_ps[:, 0:L], -2.0)
                sqj = wk.tile([128, L], F32, tag="sqj")
                musq = wk.tile([128, 1], F32, tag="musq")
                nc.scalar.activation(sqj[:], mu_ps[:, 0:L], AF.Square,
                                     accum_out=musq[:])
                sig = wk.tile([128, 1], F32, tag="sig")
                nc.scalar.activation(sig[:], mu_ps[:, L:L + 1], AF.Exp)
                nc.scalar.copy(lsg_t[:, s:s + 1], mu_ps[:, L:L + 1])
                nc.vector.tensor_tensor(out=A_t[:, s:s + 1], in0=musq[:],
                                        in1=sig[:], op=OP.add)

                mu2b = mu2[:].unsqueeze(1).to_broadcast([128, C, L])
                pc = wk.tile([128, C, L], BF16, tag="pc")
                nc.vector.tensor_tensor(out=pc[:], in0=PGs[:, :, E:E + L],
                                        in1=mu2b, op=OP.mult)
                nc.vector.tensor_reduce(out=dc[:, s * C:(s + 1) * C], in_=pc[:],
                                        axis=AX.X, op=OP.add)
                pn = wk.tile([128, C, L], BF16, tag="pn")
                nc.vector.tensor_tensor(out=pn[:], in0=NTs[:, :, 0:L],
                                        in1=mu2b, op=OP.mult)
                nc.vector.tensor_reduce(out=dn[:, s * C:(s + 1) * C], in_=pn[:],
                                        axis=AX.X, op=OP.add)
                pz = wk.tile([128, L], BF16, tag="pz")
                nc.vector.tensor_tensor(out=pz[:], in0=CG[:, s, E:E + L],
                                        in1=mu2[:], op=OP.mult)
                nc.vector.tensor_reduce(out=cd[:, s:s + 1],
                                        in_=pz[:].unsqueeze(1), axis=AX.X, op=OP.add)

            # kl: w = (A + sq + dot2)*iv + lv   (sq/iv/lv from selected tiles)
            def kl_w(dots, T, base, tg):
                vw = wk.tile([128, Q], F32, tag=tg)
                nc.vector.tensor_tensor(out=vw[:], in0=dots[:],
                                        in1=T[:, :, base + L + 1], op=OP.add)
                nc.vector.tensor_tensor(
                    out=vw[:].rearrange("p (s c) -> p s c", s=NSB),
                    in0=vw[:].rearrange("p (s c) -> p s c", s=NSB),
                    in1=A_t[:].unsqueeze(2).to_broadcast([128, NSB, C]), op=OP.add)
                nc.vector.tensor_tensor(out=vw[:], in0=vw[:],
                                        in1=T[:, :, base + L + 2], op=OP.mult)
                nc.vector.tensor_tensor(out=vw[:], in0=vw[:],
                                        in1=T[:, :, base + L], op=OP.add)
                return vw

            vwc = kl_w(dc, PG, E, "vwc")
            vwn = kl_w(dn, NT, 0, "vwn")
            d = wk.tile([128, Q], F32, tag="d")
            nc.vector.tensor_tensor(out=d[:], in0=vwc[:], in1=vwn[:], op=OP.subtract)
            hng = wk.tile([128, Q], F32, tag="hng")
            nc.scalar.activation(hng[:], d[:], AF.Relu, bias=float(MARGIN), scale=0.5)
            nc.vector.tensor_tensor(out=acc_h[:], in0=acc_h[:], in1=hng[:], op=OP.add)

            cw = wk.tile([128, NSB], F32, tag="cw")
            nc.vector.tensor_tensor(out=cw[:], in0=cd[:], in1=CG[:, :, E + L + 1], op=OP.add)
            nc.vector.tensor_tensor(out=cw[:], in0=cw[:], in1=A_t[:], op=OP.add)
            nc.vector.tensor_tensor(out=cw[:], in0=cw[:], in1=CG[:, :, E + L + 2], op=OP.mult)
            nc.vector.tensor_tensor(out=cw[:], in0=cw[:], in1=CG[:, :, E + L], op=OP.add)
            nc.vector.tensor_tensor(out=cw[:], in0=cw[:], in1=lsg_t[:], op=OP.subtract)
            nc.vector.tensor_tensor(out=acc_c[:], in0=acc_c[:], in1=cw[:], op=OP.add)

        outt = accp.tile([128, 2], F32)
        nc.vector.tensor_reduce(out=outt[:, 0:1], in_=acc_h[:], axis=AX.X, op=OP.add)
        nc.vector.tensor_reduce(out=outt[:, 1:2], in_=acc_c[:], axis=AX.X, op=OP.add)
        nc.sync.dma_start(out_d.ap(), outt[:])

    # Spread gathers across the 4 SWDGE queues (4 Q7 core-pairs run desc-gen
    # in parallel). queue = Tile-assigned DMASW sem lane % 4 keeps per-lane
    # completion FIFO within its queue, so Tile's sem ordering stays sound.
    import re
    for inst in nc.inst_map.values():
        if type(inst).__name__ == "InstDMAGatherAnt" and inst.sync_info:
            for u in inst.sync_info.on_update:
                m = re.match(r"DMASW(\d+)_", u.ant_name or "")
                if m:
                    inst.queue_num = int(m.group(1)) % 4
                    break
    nc.compile()
    return nc


def _prep_inputs(emb, W1, b1, Wmu, bmu, Wls, bls, type_means_tbl,
                 type_logvars_tbl, centers, contexts, neg_contexts):
    emb = np.asarray(emb, np.float32)
    W1 = np.asarray(W1, np.float32)
    U = emb @ W1[:D]
    Ucen = emb @ W1[D:] + np.asarray(b1, np.float32)

    tm = np.asarray(type_means_tbl, np.float32)
    lv = np.asarray(type_logvars_tbl, np.float32)[:, 0]
    sq = (tm * tm).sum(axis=1)
    iv = np.exp(-lv)

    gt = np.zeros((V, 2 * E), np.float32); gt[:, 0:D] = U
    gt[:, E:E + L] = tm; gt[:, E + L] = lv; gt[:, E + L + 1] = sq; gt[:, E + L + 2] = iv
    cg = np.zeros((V, 2 * E), np.float32); cg[:, 0:D] = Ucen
    cg[:, E:] = gt[:, E:]
    tt = np.zeros((V, E), np.float32)
    tt[:, 0:L] = tm; tt[:, L] = lv; tt[:, L + 1] = sq; tt[:, L + 2] = iv
    gt = gt.astype(ml_dtypes.bfloat16)
    cg = cg.astype(ml_dtypes.bfloat16)
    tt = tt.astype(ml_dtypes.bfloat16)

    wf = np.zeros((H + 1, L + 1), np.float32)
    wf[0:H, 0:L] = np.asarray(Wmu, np.float32)
    wf[0:H, L] = np.asarray(Wls, np.float32)[:, 0]
    wf[H, 0:L] = np.asarray(bmu, np.float32)
    wf[H, L] = np.asarray(bls, np.float32)[0]

    # flat gather order: position i = slot*128 + p; slot = s*C + j for ctx/neg,
    # slot = s for cen; b = core*NB + gb*GBS + s*128 + p
    cx = np.asarray(contexts, np.int32).reshape(NCORES, NGB, NSB, 128, C)
    ng = np.asarray(neg_contexts, np.int32).reshape(NCORES, NGB, NSB, 128, C)
    cn = np.asarray(centers, np.int32).reshape(NCORES, NGB, NSB, 128)
    # -> [core, gb, slot(s,j), p] flat per stream
    cxf = cx.transpose(0, 1, 2, 4, 3).reshape(NCORES, NGB, Q * 128)
    ngf = ng.transpose(0, 1, 2, 4, 3).reshape(NCORES, NGB, Q * 128)
    cnf = cn.reshape(NCORES, NGB, NSB * 128)

    in_maps = []
    for c in range(NCORES):
        iparts, mparts = [], []
        for gb in range(NGB):
            for f in (cxf[c, gb], ngf[c, gb], cnf[c, gb]):
                iparts.append(_wrap_idx((f >> 1).astype(np.int16)))
            # masks in [p, slot] layout
            mparts.append(np.ascontiguousarray(
                (cxf[c, gb] & 1).reshape(Q, 128).T.astype(np.uint8)))
            mparts.append(np.ascontiguousarray(
                (ngf[c, gb] & 1).reshape(Q, 128).T.astype(np.uint8)))
            mparts.append(np.ascontiguousarray(
                (cnf[c, gb] & 1).reshape(NSB, 128).T.astype(np.uint8)))
        in_maps.append({
            "gt": gt, "cg": cg, "tt": tt, "wf": wf,
            "idx": np.concatenate(iparts, axis=1),
            "msk": np.concatenate(mparts, axis=1),
        })
    return in_maps


def kernel(**inputs) -> np.ndarray:
    if "nc" not in _CACHE:
        _CACHE["nc"] = _build_program()
    nc = _CACHE["nc"]
    in_maps = _prep_inputs(**inputs)
    res = run_bass_kernel_spmd(nc, in_maps, core_ids=list(range(NCORES)))
    total = 0.0
    for c in range(NCORES):
        out = np.asarray(res.results[c]["out"], np.float64)
        total += out[:, 0].sum() + 0.5 * out[:, 1].sum()
    loss = total / B - L / 2.0
    return np.float32(loss)



# revision 4
# speedup vs baseline: 1.3215x; 1.3215x over previous
"""Trainium2 Bass kernel for the BSG word2gauss-style hinge/KL loss.

Strategy (data-parallel over 8 NeuronCores):
  - Host precomputes gather tables (batch-independent weight prep), bf16.
    Key algebra: 2*kl + L = A_b*iv_w + h'_b . g'_w + c_w - lsg_b with
      A_b  = exp(lsg_b) + sum(mu_b^2)
      g'_w = -2*iv_w*(wf[:, :L] @ tm_w)  in R^{H+1}   (projected type mean)
      c_w  = sq_w*iv_w + lv_w
    so the per-(row, word) interaction is a 51-dim dot with h' = [h; 1]
    instead of a 100-dim dot with mu, and no mu2 scaling pass is needed.
    Tables:
      CT [V,128] bf16 (256B rows): 0:50 U = emb@W1[:50], 50:101 g', 101 iv,
         102 c                                         (context stream)
      NT [V, 64] bf16 (128B rows): 0:51 g', 51 iv, 52 c     (neg stream)
      ZT [V,128] bf16: 0:50 Ucen = emb@W1[50:]+b1, rest as CT  (centers)
  - Gathers use dma_gather (SWDGE). Its int16 index limit (<32768 rows) is
    handled by gathering PAIRED rows: index = id>>1 with elem_size = 2 rows,
    then one contiguous parity select keeps the useful low columns.
    <=1024 indices per instruction (SWDGE descriptor-ring capacity), spread
    over 4 queues.
  - Each core processes 8192 batch rows in 16 gather-blocks of 512. Flat
    gather position i -> (partition i%128, slot i//128), so host index
    order is slot-major. Per 128-row sub-block:
      h = sum_j relu(U[ctx_j] + Ucen[cen]);  [h;1] @ [Wmu|Wls;bmu|bls] on PE
      A = exp(logsigma) + sum(mu^2);  dots = h' . g' for ctx/neg/cen
    then kl algebra + hinge on [128,40] vectors, accumulated in f32.
  - Output per core: [128,2] partials; host reduces, applies -L/2, /B.
"""

import sys

for _p in ("/opt/trn_rl_repo", "/opt/pypackages"):
    if _p not in sys.path:
        sys.path.append(_p)

from contextlib import ExitStack

import numpy as np
import ml_dtypes

import concourse.bass as bass
import concourse.tile as tile
from concourse import bacc, mybir
from concourse.bass_utils import run_bass_kernel_spmd
from concourse.masks import make_identity

dt = mybir.dt
F32 = dt.float32
BF16 = dt.bfloat16
AF = mybir.ActivationFunctionType
OP = mybir.AluOpType
AX = mybir.AxisListType

V, D, H, L = 50000, 50, 50, 100
C = 10
B = 65536
NCORES = 8
NB = B // NCORES     # rows per core: 8192
GBS = 512            # rows per gather block
NGB = NB // GBS      # 16
NSB = GBS // 128     # 4 sub-blocks
Q = NSB * C          # 40 ctx slots per partition per gather block
EC = 128             # CT/ZT row width (bf16 elems, 256B)
EN = 64              # NT row width (bf16 elems, 128B)
MAXI = 1024          # max idxs per dma_gather (SWDGE ring capacity)
MARGIN = 1.0
NPAY = 103           # useful bf16 cols in a CT/ZT row (U 50 + g' 51 + iv + c)
NPAYN = 53           # useful bf16 cols in an NT row (g' 51 + iv + c)

_CACHE: dict = {}


def _wrap_idx(flat):
    """int16 idx list -> [128, ceil(n/16)] wrapped-16, replicated across cores."""
    n = len(flat)
    nf = -(-n // 16)
    w = np.zeros((16, nf), np.int16)
    w[np.arange(n) % 16, np.arange(n) // 16] = flat
    return np.tile(w, (8, 1))


def _build_program():
    nc = bacc.Bacc("TRN2", target_bir_lowering=False, debug=False, num_swdge_queues=4)

    ct_d = nc.dram_tensor("ct", [V, EC], BF16, kind="ExternalInput")
    nt_d = nc.dram_tensor("nt", [V, EN], BF16, kind="ExternalInput")
    zt_d = nc.dram_tensor("zt", [V, EC], BF16, kind="ExternalInput")
    wf_d = nc.dram_tensor("wf", [H + 1, L + 1], F32, kind="ExternalInput")
    # wrapped int16 half-indices, concatenated per gather block:
    #   per gb: ctx (Q*128/16 cols) | neg | cen (NSB*128/16 cols)
    IGC = Q * 128 // 16          # 320 idx cols per gb for ctx/neg streams
    IGZ = NSB * 128 // 16        # 32 idx cols per gb for cen stream
    IG = 2 * IGC + IGZ
    idx_d = nc.dram_tensor("idx", [128, NGB * IG], dt.int16, kind="ExternalInput")
    # parity masks (uint8 0/1): per gb: ctx Q | neg Q | cen NSB
    MG = 2 * Q + NSB
    msk_d = nc.dram_tensor("msk", [128, NGB * MG], dt.uint8, kind="ExternalInput")
    out_d = nc.dram_tensor("out", [128, 2], F32, kind="ExternalOutput")

    # paired views: half-row index k -> rows [2k, 2k+1]
    ct_v = bass.AP(ct_d, 0, [[2 * EC, V // 2], [1, 2 * EC]])
    nt_v = bass.AP(nt_d, 0, [[2 * EN, V // 2], [1, 2 * EN]])
    zt_v = bass.AP(zt_d, 0, [[2 * EC, V // 2], [1, 2 * EC]])

    def gather(out_ap, tab_v, idx_ap, n, es):
        nc.gpsimd.dma_gather(
            out_ap=out_ap, in_ap=tab_v, idxs_ap=idx_ap,
            num_idxs=n, num_idxs_reg=n, elem_size=es, elem_step=es,
            queue_num=0)

    with tile.TileContext(nc) as tc, ExitStack() as ctx:
        const = ctx.enter_context(tc.tile_pool(name="const", bufs=1))
        io = ctx.enter_context(tc.tile_pool(name="io", bufs=3))
        wk = ctx.enter_context(tc.tile_pool(name="wk", bufs=2))
        ps = ctx.enter_context(tc.tile_pool(name="ps", bufs=2, space="PSUM"))
        accp = ctx.enter_context(tc.tile_pool(name="accp", bufs=1))

        ident = const.tile([128, 128], F32)
        make_identity(nc, ident[:])
        wf_sb = const.tile([H + 1, L + 1], F32)
        nc.sync.dma_start(wf_sb[:], wf_d.ap())
        idx_sb = const.tile([128, NGB * IG], dt.int16)
        nc.sync.dma_start(idx_sb[:], idx_d.ap())
        msk_sb = const.tile([128, NGB * MG], dt.uint8)
        nc.sync.dma_start(msk_sb[:], msk_d.ap())

        acc_h = accp.tile([128, Q], F32)
        acc_c = accp.tile([128, NSB], F32)
        nc.vector.memset(acc_h[:], 0.0)
        nc.vector.memset(acc_c[:], 0.0)

        for gb in range(NGB):
            PG = io.tile([128, Q, 2 * EC], BF16, tag="PG")    # ctx row pairs
            NG = io.tile([128, Q, 2 * EN], BF16, tag="NG")    # neg row pairs
            CG = io.tile([128, NSB, 2 * EC], BF16, tag="CG")  # cen row pairs

            icx = idx_sb[:, gb * IG:gb * IG + IGC]
            ing = idx_sb[:, gb * IG + IGC:gb * IG + 2 * IGC]
            icn = idx_sb[:, gb * IG + 2 * IGC:(gb + 1) * IG]
            # 1024-idx chunks: chunk g covers slots [g*8, g*8+8)
            NCH = Q * 128 // MAXI                          # 5
            SCH = MAXI // 128                              # 8 slots per chunk
            for g in range(NCH):
                sl = slice(g * SCH, (g + 1) * SCH)
                gather(PG[:, sl, :], ct_v, icx[:, g * 64:(g + 1) * 64], MAXI, 2 * EC)
                gather(NG[:, sl, :], nt_v, ing[:, g * 64:(g + 1) * 64], MAXI, 2 * EN)
            gather(CG[:], zt_v, icn, NSB * 128, 2 * EC)

            # parity select, in place: keep the chosen row in cols [0:NPAY)
            mc = msk_sb[:, gb * MG:gb * MG + Q]
            mn = msk_sb[:, gb * MG + Q:gb * MG + 2 * Q]
            mz = msk_sb[:, gb * MG + 2 * Q:(gb + 1) * MG]
            nc.vector.copy_predicated(PG[:, :, 0:NPAY],
                                      mc.unsqueeze(2).to_broadcast([128, Q, NPAY]),
                                      PG[:, :, EC:EC + NPAY])
            nc.vector.copy_predicated(NG[:, :, 0:NPAYN],
                                      mn.unsqueeze(2).to_broadcast([128, Q, NPAYN]),
                                      NG[:, :, EN:EN + NPAYN])
            nc.vector.copy_predicated(CG[:, :, 0:NPAY],
                                      mz.unsqueeze(2).to_broadcast([128, NSB, NPAY]),
                                      CG[:, :, EC:EC + NPAY])

            dc = wk.tile([128, Q], F32, tag="dc")
            dn = wk.tile([128, Q], F32, tag="dn")
            cd = wk.tile([128, NSB], F32, tag="cd")
            A_t = wk.tile([128, NSB], F32, tag="A")
            lsg_t = wk.tile([128, NSB], F32, tag="lsg")

            for s in range(NSB):
                PGs = PG[:, s * C:(s + 1) * C, :]
                NGs = NG[:, s * C:(s + 1) * C, :]

                y = wk.tile([128, C, D], BF16, tag="y")
                nc.vector.tensor_tensor(
                    out=y[:], in0=PGs[:, :, 0:D],
                    in1=CG[:, s, 0:D].unsqueeze(1).to_broadcast([128, C, D]),
                    op=OP.add)
                r = wk.tile([128, C, D], BF16, tag="r")
                nc.scalar.activation(r[:], y[:], AF.Relu)
                h = wk.tile([128, H + 1], F32, tag="h")
                nc.vector.tensor_reduce(out=h[:, 0:D], in_=r[:].transpose([0, 2, 1]),
                                        axis=AX.X, op=OP.add)
                nc.vector.memset(h[:, H:H + 1], 1.0)
                hb = wk.tile([128, H + 1], BF16, tag="hb")
                nc.scalar.copy(hb[:], h[:])
                hT_ps = ps.tile([64, 128], F32, tag="hTp")
                nc.tensor.transpose(hT_ps[0:H + 1, :], h[:], ident[:])
                hT = wk.tile([64, 128], F32, tag="hT")
                nc.scalar.copy(hT[0:H + 1, :], hT_ps[0:H + 1, :])
                mu_ps = ps.tile([128, L + 1], F32, tag="mu")
                nc.tensor.matmul(mu_ps[:], lhsT=hT[0:H + 1, :], rhs=wf_sb[:],
                                 start=True, stop=True)
                sqj = wk.tile([128, L], F32, tag="sqj")
                musq = wk.tile([128, 1], F32, tag="musq")
                nc.scalar.activation(sqj[:], mu_ps[:, 0:L], AF.Square,
                                     accum_out=musq[:])
                sig = wk.tile([128, 1], F32, tag="sig")
                nc.scalar.activation(sig[:], mu_ps[:, L:L + 1], AF.Exp)
                nc.scalar.copy(lsg_t[:, s:s + 1], mu_ps[:, L:L + 1])
                nc.vector.tensor_tensor(out=A_t[:, s:s + 1], in0=musq[:],
                                        in1=sig[:], op=OP.add)

                hbb = hb[:].unsqueeze(1).to_broadcast([128, C, H + 1])
                pc = wk.tile([128, C, H + 1], BF16, tag="pc")
                nc.vector.tensor_tensor(out=pc[:], in0=PGs[:, :, D:D + H + 1],
                                        in1=hbb, op=OP.mult)
                nc.vector.tensor_reduce(out=dc[:, s * C:(s + 1) * C], in_=pc[:],
                                        axis=AX.X, op=OP.add)
                pn = wk.tile([128, C, H + 1], BF16, tag="pn")
                nc.vector.tensor_tensor(out=pn[:], in0=NGs[:, :, 0:H + 1],
                                        in1=hbb, op=OP.mult)
                nc.vector.tensor_reduce(out=dn[:, s * C:(s + 1) * C], in_=pn[:],
                                        axis=AX.X, op=OP.add)
                pz = wk.tile([128, H + 1], BF16, tag="pz")
                nc.vector.tensor_tensor(out=pz[:], in0=CG[:, s, D:D + H + 1],
                                        in1=hb[:], op=OP.mult)
                nc.vector.tensor_reduce(out=cd[:, s:s + 1],
                                        in_=pz[:].unsqueeze(1), axis=AX.X, op=OP.add)

            # kl: vw = dots + c + A*iv
            def kl_w(dots, ivap, cap, tg):
                vw = wk.tile([128, Q], F32, tag=tg)
                nc.vector.tensor_tensor(out=vw[:], in0=dots[:], in1=cap, op=OP.add)
                av = wk.tile([128, Q], F32, tag=tg + "a")
                nc.vector.tensor_tensor(
                    out=av[:].rearrange("p (s c) -> p s c", s=NSB),
                    in0=ivap.rearrange("p (s c) -> p s c", s=NSB),
                    in1=A_t[:].unsqueeze(2).to_broadcast([128, NSB, C]), op=OP.mult)
                nc.vector.tensor_tensor(out=vw[:], in0=vw[:], in1=av[:], op=OP.add)
                return vw

            vwc = kl_w(dc, PG[:, :, D + H + 1], PG[:, :, D + H + 2], "vwc")
            vwn = kl_w(dn, NG[:, :, H + 1], NG[:, :, H + 2], "vwn")
            d = wk.tile([128, Q], F32, tag="d")
            nc.vector.tensor_tensor(out=d[:], in0=vwc[:], in1=vwn[:], op=OP.subtract)
            hng = wk.tile([128, Q], F32, tag="hng")
            nc.scalar.activation(hng[:], d[:], AF.Relu, bias=float(MARGIN), scale=0.5)
            nc.vector.tensor_tensor(out=acc_h[:], in0=acc_h[:], in1=hng[:], op=OP.add)

            cw = wk.tile([128, NSB], F32, tag="cw")
            nc.vector.tensor_tensor(out=cw[:], in0=cd[:], in1=CG[:, :, D + H + 2],
                                    op=OP.add)
            ca = wk.tile([128, NSB], F32, tag="ca")
            nc.vector.tensor_tensor(out=ca[:], in0=CG[:, :, D + H + 1], in1=A_t[:],
                                    op=OP.mult)
            nc.vector.tensor_tensor(out=cw[:], in0=cw[:], in1=ca[:], op=OP.add)
            nc.vector.tensor_tensor(out=cw[:], in0=cw[:], in1=lsg_t[:], op=OP.subtract)
            nc.vector.tensor_tensor(out=acc_c[:], in0=acc_c[:], in1=cw[:], op=OP.add)

        outt = accp.tile([128, 2], F32)
        nc.vector.tensor_reduce(out=outt[:, 0:1], in_=acc_h[:], axis=AX.X, op=OP.add)
        nc.vector.tensor_reduce(out=outt[:, 1:2], in_=acc_c[:], axis=AX.X, op=OP.add)
        nc.sync.dma_start(out_d.ap(), outt[:])

    # Spread gathers across the 4 SWDGE queues (4 Q7 core-pairs run desc-gen
    # in parallel). queue = Tile-assigned DMASW sem lane % 4 keeps per-lane
    # completion FIFO within its queue, so Tile's sem ordering stays sound.
    import re
    for inst in nc.inst_map.values():
        if type(inst).__name__ == "InstDMAGatherAnt" and inst.sync_info:
            for u in inst.sync_info.on_update:
                m = re.match(r"DMASW(\d+)_", u.ant_name or "")
                if m:
                    inst.queue_num = int(m.group(1)) % 4
                    break
    nc.compile()
    return nc


def _prep_inputs(emb, W1, b1, Wmu, bmu, Wls, bls, type_means_tbl,
                 type_logvars_tbl, centers, contexts, neg_contexts):
    emb = np.asarray(emb, np.float32)
    W1 = np.asarray(W1, np.float32)
    U = emb @ W1[:D]
    Ucen = emb @ W1[D:] + np.asarray(b1, np.float32)

    tm = np.asarray(type_means_tbl, np.float32)
    lv = np.asarray(type_logvars_tbl, np.float32)[:, 0]
    sq = (tm * tm).sum(axis=1)
    iv = np.exp(-lv)

    wf = np.zeros((H + 1, L + 1), np.float32)
    wf[0:H, 0:L] = np.asarray(Wmu, np.float32)
    wf[0:H, L] = np.asarray(Wls, np.float32)[:, 0]
    wf[H, 0:L] = np.asarray(bmu, np.float32)
    wf[H, L] = np.asarray(bls, np.float32)[0]

    G = (tm @ wf[0:H + 1, 0:L].T) * (-2.0 * iv)[:, None]    # [V, H+1]
    c = sq * iv + lv

    ct = np.zeros((V, EC), np.float32)
    ct[:, 0:D] = U
    ct[:, D:D + H + 1] = G
    ct[:, D + H + 1] = iv
    ct[:, D + H + 2] = c
    zt = ct.copy()
    zt[:, 0:D] = Ucen
    nt = np.zeros((V, EN), np.float32)
    nt[:, 0:H + 1] = G
    nt[:, H + 1] = iv
    nt[:, H + 2] = c
    ct = ct.astype(ml_dtypes.bfloat16)
    zt = zt.astype(ml_dtypes.bfloat16)
    nt = nt.astype(ml_dtypes.bfloat16)

    # flat gather order: position i = slot*128 + p; slot = s*C + j for ctx/neg,
    # slot = s for cen; b = core*NB + gb*GBS + s*128 + p
    cx = np.asarray(contexts, np.int32).reshape(NCORES, NGB, NSB, 128, C)
    ng = np.asarray(neg_contexts, np.int32).reshape(NCORES, NGB, NSB, 128, C)
    cn = np.asarray(centers, np.int32).reshape(NCORES, NGB, NSB, 128)
    # -> [core, gb, slot(s,j), p] flat per stream
    cxf = cx.transpose(0, 1, 2, 4, 3).reshape(NCORES, NGB, Q * 128)
    ngf = ng.transpose(0, 1, 2, 4, 3).reshape(NCORES, NGB, Q * 128)
    cnf = cn.reshape(NCORES, NGB, NSB * 128)

    in_maps = []
    for cix in range(NCORES):
        iparts, mparts = [], []
        for gb in range(NGB):
            for f in (cxf[cix, gb], ngf[cix, gb], cnf[cix, gb]):
                iparts.append(_wrap_idx((f >> 1).astype(np.int16)))
            # masks in [p, slot] layout
            mparts.append(np.ascontiguousarray(
                (cxf[cix, gb] & 1).reshape(Q, 128).T.astype(np.uint8)))
            mparts.append(np.ascontiguousarray(
                (ngf[cix, gb] & 1).reshape(Q, 128).T.astype(np.uint8)))
            mparts.append(np.ascontiguousarray(
                (cnf[cix, gb] & 1).reshape(NSB, 128).T.astype(np.uint8)))
        in_maps.append({
            "ct": ct, "nt": nt, "zt": zt, "wf": wf,
            "idx": np.concatenate(iparts, axis=1),
            "msk": np.concatenate(mparts, axis=1),
        })
    return in_maps


def kernel(**inputs) -> np.ndarray:
    if "nc" not in _CACHE:
        _CACHE["nc"] = _build_program()
    nc = _CACHE["nc"]
    in_maps = _prep_inputs(**inputs)
    res = run_bass_kernel_spmd(nc, in_maps, core_ids=list(range(NCORES)))
    total = 0.0
    for cix in range(NCORES):
        out = np.asarray(res.results[cix]["out"], np.float64)
        total += out[:, 0].sum() + 0.5 * out[:, 1].sum()
    loss = total / B - L / 2.0
    return np.float32(loss)


# revision 5
# speedup vs baseline: 1.7510x; 1.3250x over previous
"""Trainium2 Bass kernel for the BSG word2gauss-style hinge/KL loss.

Strategy (data-parallel over 8 NeuronCores):
  - Host precomputes gather tables (batch-independent weight prep), bf16.
    Key algebra: 2*kl + L = A_b*iv_w + h'_b . g'_w + c_w - lsg_b with
      A_b  = exp(lsg_b) + sum(mu_b^2)
      g'_w = -2*iv_w*(wf[:, :L] @ tm_w)  in R^{H+1}   (projected type mean)
      c_w  = sq_w*iv_w + lv_w
    so the per-(row, word) interaction is a 51-dim dot with h' = [h; 1]
    instead of a 100-dim dot with mu, and no mu2 scaling pass is needed.
    Tables:
      CT [V,128] bf16 (256B rows): 0:50 U = emb@W1[:50], 50:101 g', 101 iv,
         102 c                                         (context stream)
      NT [V, 64] bf16 (128B rows): 0:51 g', 51 iv, 52 c     (neg stream)
      ZT [V,128] bf16: 0:50 Ucen = emb@W1[50:]+b1, rest as CT  (centers)
  - Gathers use dma_gather (SWDGE). Its int16 index limit (<32768 rows) is
    handled by gathering PAIRED rows: index = id>>1 with elem_size = 2 rows,
    then one contiguous parity select keeps the useful low columns.
    <=1024 indices per instruction (SWDGE descriptor-ring capacity), spread
    over 4 queues.
  - Each core processes 8192 batch rows in 16 gather-blocks of 512. Flat
    gather position i -> (partition i%128, slot i//128), so host index
    order is slot-major. Per 128-row sub-block:
      h = sum_j relu(U[ctx_j] + Ucen[cen]);  [h;1] @ [Wmu|Wls;bmu|bls] on PE
      A = exp(logsigma) + sum(mu^2);  dots = h' . g' for ctx/neg/cen
    then kl algebra + hinge on [128,40] vectors, accumulated in f32.
  - Output per core: [128,2] partials; host reduces, applies -L/2, /B.
"""

import sys

for _p in ("/opt/trn_rl_repo", "/opt/pypackages"):
    if _p not in sys.path:
        sys.path.append(_p)

from contextlib import ExitStack

import numpy as np
import ml_dtypes

import concourse.bass as bass
import concourse.tile as tile
from concourse import bacc, mybir
from concourse.bass_utils import run_bass_kernel_spmd
from concourse.masks import make_identity

dt = mybir.dt
F32 = dt.float32
BF16 = dt.bfloat16
AF = mybir.ActivationFunctionType
OP = mybir.AluOpType
AX = mybir.AxisListType

V, D, H, L = 50000, 50, 50, 100
C = 10
B = 65536
NCORES = 8
NB = B // NCORES     # rows per core: 8192
GBS = 512            # rows per gather block
NGB = NB // GBS      # 16
NSB = GBS // 128     # 4 sub-blocks
Q = NSB * C          # 40 ctx slots per partition per gather block
EC = 128             # CT/ZT row width (bf16 elems, 256B)
EN = 64              # NT row width (bf16 elems, 128B)
MAXI = 1024          # max idxs per dma_gather (SWDGE ring capacity)
MARGIN = 1.0
NPAY = 103           # useful bf16 cols in a CT/ZT row (U 50 + g' 51 + iv + c)
NPAYN = 53           # useful bf16 cols in an NT row (g' 51 + iv + c)

_CACHE: dict = {}


def _wrap_idx(flat):
    """int16 idx list -> [128, ceil(n/16)] wrapped-16, replicated across cores."""
    n = len(flat)
    nf = -(-n // 16)
    w = np.zeros((16, nf), np.int16)
    w[np.arange(n) % 16, np.arange(n) // 16] = flat
    return np.tile(w, (8, 1))


def _build_program():
    nc = bacc.Bacc("TRN2", target_bir_lowering=False, debug=False, num_swdge_queues=4)

    ct_d = nc.dram_tensor("ct", [V, EC], BF16, kind="ExternalInput")
    nt_d = nc.dram_tensor("nt", [V, EN], BF16, kind="ExternalInput")
    zt_d = nc.dram_tensor("zt", [V, EC], BF16, kind="ExternalInput")
    wf_d = nc.dram_tensor("wf", [H + 1, L + 1], F32, kind="ExternalInput")
    # wrapped int16 half-indices, concatenated per gather block:
    #   per gb: ctx (Q*128/16 cols) | neg | cen (NSB*128/16 cols)
    IGC = Q * 128 // 16          # 320 idx cols per gb for ctx/neg streams
    IGZ = NSB * 128 // 16        # 32 idx cols per gb for cen stream
    IG = 2 * IGC + IGZ
    idx_d = nc.dram_tensor("idx", [128, NGB * IG], dt.int16, kind="ExternalInput")
    # parity masks (uint8 0/1): per gb: ctx Q | neg Q | cen NSB
    MG = 2 * Q + NSB
    msk_d = nc.dram_tensor("msk", [128, NGB * MG], dt.uint8, kind="ExternalInput")
    out_d = nc.dram_tensor("out", [128, 2], F32, kind="ExternalOutput")

    # paired views: half-row index k -> rows [2k, 2k+1]
    ct_v = bass.AP(ct_d, 0, [[2 * EC, V // 2], [1, 2 * EC]])
    nt_v = bass.AP(nt_d, 0, [[2 * EN, V // 2], [1, 2 * EN]])
    zt_v = bass.AP(zt_d, 0, [[2 * EC, V // 2], [1, 2 * EC]])

    def gather(out_ap, tab_v, idx_ap, n, es):
        nc.gpsimd.dma_gather(
            out_ap=out_ap, in_ap=tab_v, idxs_ap=idx_ap,
            num_idxs=n, num_idxs_reg=n, elem_size=es, elem_step=es,
            queue_num=0)

    with tile.TileContext(nc) as tc, ExitStack() as ctx:
        const = ctx.enter_context(tc.tile_pool(name="const", bufs=1))
        io = ctx.enter_context(tc.tile_pool(name="io", bufs=3))
        wk = ctx.enter_context(tc.tile_pool(name="wk", bufs=2))
        ps = ctx.enter_context(tc.tile_pool(name="ps", bufs=2, space="PSUM"))
        accp = ctx.enter_context(tc.tile_pool(name="accp", bufs=1))

        ident = const.tile([128, 128], F32)
        make_identity(nc, ident[:])
        wf_sb = const.tile([H + 1, L + 1], F32)
        nc.sync.dma_start(wf_sb[:], wf_d.ap())
        idx_sb = const.tile([128, NGB * IG], dt.int16)
        nc.sync.dma_start(idx_sb[:], idx_d.ap())
        msk_sb = const.tile([128, NGB * MG], dt.uint8)
        nc.sync.dma_start(msk_sb[:], msk_d.ap())

        acc_h = accp.tile([128, Q], F32)
        acc_c = accp.tile([128, NSB], F32)
        nc.vector.memset(acc_h[:], 0.0)
        nc.vector.memset(acc_c[:], 0.0)

        for gb in range(NGB):
            PG = io.tile([128, Q, 2 * EC], BF16, tag="PG")    # ctx row pairs
            NG = io.tile([128, Q, 2 * EN], BF16, tag="NG")    # neg row pairs
            CG = io.tile([128, NSB, 2 * EC], BF16, tag="CG")  # cen row pairs

            icx = idx_sb[:, gb * IG:gb * IG + IGC]
            ing = idx_sb[:, gb * IG + IGC:gb * IG + 2 * IGC]
            icn = idx_sb[:, gb * IG + 2 * IGC:(gb + 1) * IG]
            # 1024-idx chunks: chunk g covers slots [g*8, g*8+8)
            NCH = Q * 128 // MAXI                          # 5
            SCH = MAXI // 128                              # 8 slots per chunk
            for g in range(NCH):
                sl = slice(g * SCH, (g + 1) * SCH)
                gather(PG[:, sl, :], ct_v, icx[:, g * 64:(g + 1) * 64], MAXI, 2 * EC)
                gather(NG[:, sl, :], nt_v, ing[:, g * 64:(g + 1) * 64], MAXI, 2 * EN)
            gather(CG[:], zt_v, icn, NSB * 128, 2 * EC)

            # parity select, in place, on f32-bitcast views (half the lanes):
            # keep the chosen row in cols [0:NPAY)
            mc = msk_sb[:, gb * MG:gb * MG + Q]
            mn = msk_sb[:, gb * MG + Q:gb * MG + 2 * Q]
            mz = msk_sb[:, gb * MG + 2 * Q:(gb + 1) * MG]
            NP2, NPN2 = (NPAY + 1) // 2, (NPAYN + 1) // 2
            nc.vector.copy_predicated(PG[:, :, 0:2 * NP2].bitcast(F32),
                                      mc.unsqueeze(2).to_broadcast([128, Q, NP2]),
                                      PG[:, :, EC:EC + 2 * NP2].bitcast(F32))
            nc.vector.copy_predicated(NG[:, :, 0:2 * NPN2].bitcast(F32),
                                      mn.unsqueeze(2).to_broadcast([128, Q, NPN2]),
                                      NG[:, :, EN:EN + 2 * NPN2].bitcast(F32))
            nc.vector.copy_predicated(CG[:, :, 0:2 * NP2].bitcast(F32),
                                      mz.unsqueeze(2).to_broadcast([128, NSB, NP2]),
                                      CG[:, :, EC:EC + 2 * NP2].bitcast(F32))

            PG4 = PG[:].rearrange("p (s c) e -> p s c e", s=NSB)
            NG4 = NG[:].rearrange("p (s c) e -> p s c e", s=NSB)

            # h = sum_j relu(U_ctx + U_cen), one batched pass per block
            y4 = wk.tile([128, NSB, C, D], BF16, tag="y4")
            nc.vector.tensor_tensor(
                out=y4[:], in0=PG4[:, :, :, 0:D],
                in1=CG[:, :, 0:D].unsqueeze(2).to_broadcast([128, NSB, C, D]),
                op=OP.add)
            r4 = wk.tile([128, NSB, D, C], BF16, tag="r4")
            nc.scalar.activation(r4[:].transpose([0, 1, 3, 2]), y4[:], AF.Relu)
            h4 = wk.tile([128, NSB, H + 1], F32, tag="h4")
            nc.vector.tensor_reduce(out=h4[:, :, 0:D], in_=r4[:],
                                    axis=AX.X, op=OP.add)
            nc.vector.memset(h4[:, :, H:H + 1], 1.0)
            hb4 = wk.tile([128, NSB, H + 1], BF16, tag="hb4")
            nc.scalar.copy(hb4[:], h4[:])

            # mu = h' @ wf on PE, per sub-block; batched epilogue
            hT_ps = ps.tile([64, NSB * 128], F32, tag="hTp")
            for s in range(NSB):
                nc.tensor.transpose(hT_ps[0:H + 1, s * 128:(s + 1) * 128],
                                    h4[:, s, :], ident[:])
            hT = wk.tile([64, NSB * 128], F32, tag="hT")
            nc.scalar.copy(hT[0:H + 1, :], hT_ps[0:H + 1, :])
            mu_ps = ps.tile([128, NSB, L + 1], F32, tag="mu")
            for s in range(NSB):
                nc.tensor.matmul(mu_ps[:, s, :],
                                 lhsT=hT[0:H + 1, s * 128:(s + 1) * 128],
                                 rhs=wf_sb[:], start=True, stop=True)
            A_t = wk.tile([128, NSB], F32, tag="A")
            sqj = wk.tile([128, L], F32, tag="sqj")
            for s in range(NSB):
                nc.scalar.activation(sqj[:], mu_ps[:, s, 0:L], AF.Square,
                                     accum_out=A_t[:, s:s + 1])
            sig = wk.tile([128, NSB], F32, tag="sig")
            nc.scalar.activation(sig[:], mu_ps[:, :, L], AF.Exp)
            lsg_t = wk.tile([128, NSB], F32, tag="lsg")
            nc.scalar.copy(lsg_t[:], mu_ps[:, :, L])
            nc.vector.tensor_tensor(out=A_t[:], in0=A_t[:], in1=sig[:], op=OP.add)

            # dots = h' . g' for ctx/neg/cen, one batched mult+reduce each
            hbb = hb4[:].unsqueeze(2).to_broadcast([128, NSB, C, H + 1])
            dc = wk.tile([128, NSB, C], F32, tag="dc")
            dn = wk.tile([128, NSB, C], F32, tag="dn")
            cd = wk.tile([128, NSB], F32, tag="cd")
            pc = wk.tile([128, NSB, C, H + 1], BF16, tag="pc")
            nc.vector.tensor_tensor(out=pc[:], in0=PG4[:, :, :, D:D + H + 1],
                                    in1=hbb, op=OP.mult)
            nc.vector.tensor_reduce(out=dc[:], in_=pc[:], axis=AX.X, op=OP.add)
            pn = wk.tile([128, NSB, C, H + 1], BF16, tag="pn")
            nc.vector.tensor_tensor(out=pn[:], in0=NG4[:, :, :, 0:H + 1],
                                    in1=hbb, op=OP.mult)
            nc.vector.tensor_reduce(out=dn[:], in_=pn[:], axis=AX.X, op=OP.add)
            pz = wk.tile([128, NSB, H + 1], BF16, tag="pz")
            nc.vector.tensor_tensor(out=pz[:], in0=CG[:, :, D:D + H + 1],
                                    in1=hb4[:], op=OP.mult)
            nc.vector.tensor_reduce(out=cd[:], in_=pz[:], axis=AX.X, op=OP.add)

            # hinge: d = (dc-dn) + (cc-cn) + A*(ivc-ivn); relu(0.5*d + 1)
            v1 = wk.tile([128, NSB, C], F32, tag="v1")
            nc.vector.tensor_tensor(out=v1[:], in0=dc[:], in1=dn[:], op=OP.subtract)
            v2 = wk.tile([128, NSB, C], F32, tag="v2")
            nc.vector.tensor_tensor(out=v2[:], in0=PG4[:, :, :, D + H + 2],
                                    in1=NG4[:, :, :, H + 2], op=OP.subtract)
            v3 = wk.tile([128, NSB, C], F32, tag="v3")
            nc.vector.tensor_tensor(out=v3[:], in0=PG4[:, :, :, D + H + 1],
                                    in1=NG4[:, :, :, H + 1], op=OP.subtract)
            nc.vector.tensor_tensor(
                out=v3[:], in0=v3[:],
                in1=A_t[:].unsqueeze(2).to_broadcast([128, NSB, C]), op=OP.mult)
            nc.vector.tensor_tensor(out=v1[:], in0=v1[:], in1=v2[:], op=OP.add)
            nc.vector.tensor_tensor(out=v1[:], in0=v1[:], in1=v3[:], op=OP.add)
            hng = wk.tile([128, Q], F32, tag="hng")
            nc.scalar.activation(hng[:].rearrange("p (s c) -> p s c", s=NSB), v1[:],
                                 AF.Relu, bias=float(MARGIN), scale=0.5)
            nc.vector.tensor_tensor(out=acc_h[:], in0=acc_h[:], in1=hng[:], op=OP.add)

            cw = wk.tile([128, NSB], F32, tag="cw")
            nc.vector.tensor_tensor(out=cw[:], in0=cd[:], in1=CG[:, :, D + H + 2],
                                    op=OP.add)
            ca = wk.tile([128, NSB], F32, tag="ca")
            nc.vector.tensor_tensor(out=ca[:], in0=CG[:, :, D + H + 1], in1=A_t[:],
                                    op=OP.mult)
            nc.vector.tensor_tensor(out=cw[:], in0=cw[:], in1=ca[:], op=OP.add)
            nc.vector.tensor_tensor(out=cw[:], in0=cw[:], in1=lsg_t[:], op=OP.subtract)
            nc.vector.tensor_tensor(out=acc_c[:], in0=acc_c[:], in1=cw[:], op=OP.add)

        outt = accp.tile([128, 2], F32)
        nc.vector.tensor_reduce(out=outt[:, 0:1], in_=acc_h[:], axis=AX.X, op=OP.add)
        nc.vector.tensor_reduce(out=outt[:, 1:2], in_=acc_c[:], axis=AX.X, op=OP.add)
        nc.sync.dma_start(out_d.ap(), outt[:])

    # Spread gathers across the 4 SWDGE queues (4 Q7 core-pairs run desc-gen
    # in parallel). queue = Tile-assigned DMASW sem lane % 4 keeps per-lane
    # completion FIFO within its queue, so Tile's sem ordering stays sound.
    import re
    for inst in nc.inst_map.values():
        if type(inst).__name__ == "InstDMAGatherAnt" and inst.sync_info:
            for u in inst.sync_info.on_update:
                m = re.match(r"DMASW(\d+)_", u.ant_name or "")
                if m:
                    inst.queue_num = int(m.group(1)) % 4
                    break
    nc.compile()
    return nc


def _prep_inputs(emb, W1, b1, Wmu, bmu, Wls, bls, type_means_tbl,
                 type_logvars_tbl, centers, contexts, neg_contexts):
    emb = np.asarray(emb, np.float32)
    W1 = np.asarray(W1, np.float32)
    U = emb @ W1[:D]
    Ucen = emb @ W1[D:] + np.asarray(b1, np.float32)

    tm = np.asarray(type_means_tbl, np.float32)
    lv = np.asarray(type_logvars_tbl, np.float32)[:, 0]
    sq = (tm * tm).sum(axis=1)
    iv = np.exp(-lv)

    wf = np.zeros((H + 1, L + 1), np.float32)
    wf[0:H, 0:L] = np.asarray(Wmu, np.float32)
    wf[0:H, L] = np.asarray(Wls, np.float32)[:, 0]
    wf[H, 0:L] = np.asarray(bmu, np.float32)
    wf[H, L] = np.asarray(bls, np.float32)[0]

    G = (tm @ wf[0:H + 1, 0:L].T) * (-2.0 * iv)[:, None]    # [V, H+1]
    c = sq * iv + lv

    ct = np.zeros((V, EC), np.float32)
    ct[:, 0:D] = U
    ct[:, D:D + H + 1] = G
    ct[:, D + H + 1] = iv
    ct[:, D + H + 2] = c
    zt = ct.copy()
    zt[:, 0:D] = Ucen
    nt = np.zeros((V, EN), np.float32)
    nt[:, 0:H + 1] = G
    nt[:, H + 1] = iv
    nt[:, H + 2] = c
    ct = ct.astype(ml_dtypes.bfloat16)
    zt = zt.astype(ml_dtypes.bfloat16)
    nt = nt.astype(ml_dtypes.bfloat16)

    # flat gather order: position i = slot*128 + p; slot = s*C + j for ctx/neg,
    # slot = s for cen; b = core*NB + gb*GBS + s*128 + p
    cx = np.asarray(contexts, np.int32).reshape(NCORES, NGB, NSB, 128, C)
    ng = np.asarray(neg_contexts, np.int32).reshape(NCORES, NGB, NSB, 128, C)
    cn = np.asarray(centers, np.int32).reshape(NCORES, NGB, NSB, 128)
    # -> [core, gb, slot(s,j), p] flat per stream
    cxf = cx.transpose(0, 1, 2, 4, 3).reshape(NCORES, NGB, Q * 128)
    ngf = ng.transpose(0, 1, 2, 4, 3).reshape(NCORES, NGB, Q * 128)
    cnf = cn.reshape(NCORES, NGB, NSB * 128)

    in_maps = []
    for cix in range(NCORES):
        iparts, mparts = [], []
        for gb in range(NGB):
            for f in (cxf[cix, gb], ngf[cix, gb], cnf[cix, gb]):
                iparts.append(_wrap_idx((f >> 1).astype(np.int16)))
            # masks in [p, slot] layout
            mparts.append(np.ascontiguousarray(
                (cxf[cix, gb] & 1).reshape(Q, 128).T.astype(np.uint8)))
            mparts.append(np.ascontiguousarray(
                (ngf[cix, gb] & 1).reshape(Q, 128).T.astype(np.uint8)))
            mparts.append(np.ascontiguousarray(
                (cnf[cix, gb] & 1).reshape(NSB, 128).T.astype(np.uint8)))
        in_maps.append({
            "ct": ct, "nt": nt, "zt": zt, "wf": wf,
            "idx": np.concatenate(iparts, axis=1),
            "msk": np.concatenate(mparts, axis=1),
        })
    return in_maps


def kernel(**inputs) -> np.ndarray:
    if "nc" not in _CACHE:
        _CACHE["nc"] = _build_program()
    nc = _CACHE["nc"]
    in_maps = _prep_inputs(**inputs)
    res = run_bass_kernel_spmd(nc, in_maps, core_ids=list(range(NCORES)))
    total = 0.0
    for cix in range(NCORES):
        out = np.asarray(res.results[cix]["out"], np.float64)
        total += out[:, 0].sum() + 0.5 * out[:, 1].sum()
    loss = total / B - L / 2.0
    return np.float32(loss)
